# revision 1
# baseline (speedup 1.0000x reference)
"""AttentionDecoder Trainium2 kernel — 8-core SPMD.

Strategy:
  - Data-parallel recurrence: core c owns batch slice [8c, 8c+8).
    LSTM+attention runs fully on-device; per-step gate matmuls stream the
    (replicated) recurrent weights as the moving operand in float32r.
  - One AllGather of the hidden states (bf16) after the recurrence.
  - Vocab-parallel output projection: core c computes preds[:, :, 4000c:4000c+4000]
    (bf16 matmul, fp32 accumulate), host concatenates the 8 vocab shards.

Host-side work is layout-only: shard/transpose/cast weights, fold LayerNorm
affine params and biases into adjacent matmuls. All math (LN, gather,
recurrence, attention, projection) runs on the NeuronCores.
"""

import os
import sys

sys.path.insert(0, "/opt/trn_rl_repo")

import ml_dtypes
import numpy as np

import concourse.bass as bass
from concourse import bacc
import concourse.mybir as mybir
import concourse.tile as tile
from concourse.bass_utils import run_bass_kernel_spmd
from concourse.masks import make_identity

# problem shapes (hardcoded per harness contract)
B, S, H, E, V, NL2, T = 64, 64, 512, 256, 32000, 4, 32
NCORES = 8
BL = B // NCORES  # 8 examples per core
VL = V // NCORES  # 4000 vocab rows per core
EPS = 1e-5
BS = BL * S  # 512 rows of encoder per core
TB = T * BL  # 256 (t, b) rows per core
G4 = 4 * H  # 2048 gate dim
BT = B * T  # 2048 gathered rows

F32 = mybir.dt.float32
F32R = mybir.dt.float32r
BF16 = mybir.dt.bfloat16
I32 = mybir.dt.int32
AF = mybir.ActivationFunctionType
ALU = mybir.AluOpType

bf16 = ml_dtypes.bfloat16


def _bc_free(ap, n):
    """Append a step-0 free dim of size n (broadcast along a new inner axis)."""
    return bass.AP(tensor=ap.tensor, offset=ap.offset, ap=[*ap.ap, [0, n]])


def _bc_col(ap, n):
    """[P, 1] column -> [P, n] broadcast (replace free dim with step-0)."""
    return bass.AP(tensor=ap.tensor, offset=ap.offset, ap=[ap.ap[0], [0, n]])


def build_nc(consts, debug=False):
    """Build the SPMD Bass program. consts: python-float immediates for the
    tiny init-projection weights."""
    nc = bacc.Bacc()

    # ---------------- DRAM I/O ----------------
    d_enc = nc.dram_tensor("enc", [BS, H], F32, kind="ExternalInput")
    d_ehnT = nc.dram_tensor("ehnT", [H, NL2 * BL], F32, kind="ExternalInput")
    d_ecn = nc.dram_tensor("ecn", [BL, NL2 * H], F32, kind="ExternalInput")
    d_emb = nc.dram_tensor("emb", [V, E], F32, kind="ExternalInput")
    d_tgt = nc.dram_tensor("tgt", [TB, 1], I32, kind="ExternalInput")
    d_kwT = nc.dram_tensor("kwT", [H, H], BF16, kind="ExternalInput")
    d_qwT = nc.dram_tensor("qwT", [H, H], BF16, kind="ExternalInput")
    d_ewT = nc.dram_tensor("ewT", [H, 1], BF16, kind="ExternalInput")
    d_qadd = nc.dram_tensor("qadd", [H, 1], F32, kind="ExternalInput")
    d_wzT = nc.dram_tensor("wzT", [2 * H, G4], F32R, kind="ExternalInput")
    d_xwT = nc.dram_tensor("xwT", [E + 1, G4], BF16, kind="ExternalInput")
    d_owT = nc.dram_tensor("owT", [H, VL], BF16, kind="ExternalInput")
    d_ob = nc.dram_tensor("ob", [1, VL], F32, kind="ExternalInput")
    d_out = nc.dram_tensor("out", [B, T, VL], F32, kind="ExternalOutput")

    # internal DRAM for the collective
    d_ccin = nc.dram_tensor("ccin", [H, TB], BF16)
    d_ccout = nc.dram_tensor("ccout", [NCORES * H, TB], BF16, addr_space="Shared")

    dbg = {}
    if debug:
        for nm, shp, dt_ in [
            ("dbg_encT", [H, BS], BF16), ("dbg_pkT", [H, BS], BF16),
            ("dbg_xg", [TB, G4], F32), ("dbg_h0T", [H, BL], F32),
            ("dbg_c0", [BL, H], F32), ("dbg_qT", [H, BL], BF16),
            ("dbg_tanh", [H, BS], BF16), ("dbg_alpha", [1, BS], F32),
            ("dbg_ctxT", [H, BL], F32), ("dbg_gates", [BL, G4], F32),
            ("dbg_hsT", [H, TB], F32),
            ("dbg_hsall", [H, BT], BF16),
            ("dbg_chunk", [128, 500], F32),
            ("dbg_xlnT", [E, TB], BF16),
        ]:
            dbg[nm] = nc.dram_tensor(nm, shp, dt_, kind="ExternalOutput")

    KT = H // 128  # 4 partition tiles for the 512 hidden dim

    with tile.TileContext(nc) as tc:
        with (
            tc.tile_pool(name="persist", bufs=1) as P_per,
            tc.tile_pool(name="recur", bufs=1) as P_rec,
            tc.tile_pool(name="cell", bufs=1) as P_cell,
            tc.tile_pool(name="psA", bufs=1, space="PSUM") as PS_a,
            tc.tile_pool(name="psG", bufs=2, space="PSUM") as PS_g,
        ):
            # ---------- persistent SBUF ----------
            id128 = P_per.tile([128, 128], F32, name="id128")
            make_identity(nc, id128[:, :])
            id8 = P_per.tile([8, 8], F32, name="id8")
            make_identity(nc, id8[:, :])
            eps_t = P_per.tile([128, 1], F32, name="eps")
            nc.vector.memset(eps_t[:, :], EPS)

            # hidden-state history (transposed): hs_T[kt] cols (t, b)
            hs_T = [P_per.tile([128, TB], F32R, name=f"hsT{k}") for k in range(KT)]
            hs_Tb = [P_per.tile([128, TB], BF16, name=f"hsTb{k}") for k in range(KT)]
            c_st = P_per.tile([BL, H], F32, name="c_state")

            # ---------- weights in SBUF (recurrence scope) ----------
            kwT = [P_rec.tile([128, H], BF16, name=f"kwT{k}") for k in range(KT)]
            qwT = [P_rec.tile([128, H], BF16, name=f"qwT{k}") for k in range(KT)]
            ewT = [P_rec.tile([128, 1], BF16, name=f"ewT{k}") for k in range(KT)]
            qadd = [P_rec.tile([128, 1], F32, name=f"qadd{k}") for k in range(KT)]
            wzT = [P_rec.tile([128, G4], F32R, name=f"wzT{k}") for k in range(2 * KT)]
            for k in range(KT):
                nc.sync.dma_start(kwT[k][:, :], d_kwT[128 * k : 128 * (k + 1), :])
                nc.sync.dma_start(qwT[k][:, :], d_qwT[128 * k : 128 * (k + 1), :])
                nc.sync.dma_start(ewT[k][:, :], d_ewT[128 * k : 128 * (k + 1), :])
                nc.sync.dma_start(qadd[k][:, :], d_qadd[128 * k : 128 * (k + 1), :])
            for k in range(2 * KT):
                nc.sync.dma_start(wzT[k][:, :], d_wzT[128 * k : 128 * (k + 1), :])

            enc_Tb = [P_rec.tile([128, BS], BF16, name=f"encT{k}") for k in range(KT)]
            pk_Tb = [P_rec.tile([128, BS], BF16, name=f"pkT{g}") for g in range(KT)]
            x_gates = [P_rec.tile([128, G4], F32, name=f"xg{m}") for m in range(2)]
            h0_T = [P_rec.tile([128, BL], F32R, name=f"h0T{k}") for k in range(KT)]
            h0_Tb = [P_rec.tile([128, BL], BF16, name=f"h0Tb{k}") for k in range(KT)]

            # ========== precompute (scoped pools; freed before recurrence) ==========
            with (
                tc.tile_pool(name="pre", bufs=1) as P_pre,
                tc.tile_pool(name="prew", bufs=2) as P_pw,
            ):
                xwT = [P_pre.tile([128, G4], BF16, name=f"xwT{k}") for k in range(2)]
                xwTb = P_pre.tile([1, G4], BF16, name="xwTbias")
                for k in range(2):
                    nc.sync.dma_start(xwT[k][:, :], d_xwT[128 * k : 128 * (k + 1), :])
                nc.sync.dma_start(xwTb[:, :], d_xwT[2 * 128 : 2 * 128 + 1, :])

                # ----- h0 / c0 -----
                ehnT = [
                    P_pre.tile([128, NL2 * BL], F32, name=f"ehnT{k}") for k in range(KT)
                ]
                for k in range(KT):
                    nc.sync.dma_start(ehnT[k][:, :], d_ehnT[128 * k : 128 * (k + 1), :])
                ecn = P_pre.tile([BL, NL2 * H], F32, name="ecn")
                nc.sync.dma_start(ecn[:, :], d_ecn[:, :])

                phw, pcw = consts["phw"], consts["pcw"]
                for k in range(KT):
                    tmp = P_pw.tile([128, BL], F32, name="h0tmp")
                    nc.vector.tensor_scalar_mul(
                        h0_T[k][:, :], ehnT[k][:, 0:BL], float(phw[0])
                    )
                    for l in range(1, NL2):
                        nc.vector.tensor_scalar_mul(
                            tmp[:, :], ehnT[k][:, BL * l : BL * (l + 1)], float(phw[l])
                        )
                        nc.vector.tensor_add(h0_T[k][:, :], h0_T[k][:, :], tmp[:, :])
                    nc.vector.tensor_scalar_add(
                        h0_T[k][:, :], h0_T[k][:, :], float(consts["phb"])
                    )
                    nc.vector.tensor_copy(h0_Tb[k][:, :], h0_T[k][:, :])

                ctmp = P_pw.tile([BL, H], F32, name="c0tmp")
                nc.vector.tensor_scalar_mul(c_st[:, :], ecn[:, 0:H], float(pcw[0]))
                for l in range(1, NL2):
                    nc.vector.tensor_scalar_mul(
                        ctmp[:, :], ecn[:, H * l : H * (l + 1)], float(pcw[l])
                    )
                    nc.vector.tensor_add(c_st[:, :], c_st[:, :], ctmp[:, :])
                nc.vector.tensor_scalar_add(
                    c_st[:, :], c_st[:, :], float(consts["pcb"])
                )

                # ----- encoder LN (natural layout) + transpose -----
                enc_ln = [P_pre.tile([128, H], F32, name=f"encln{i}") for i in range(4)]
                for i in range(BS // 128):
                    x_t = P_pw.tile([128, H], F32, name="enc_in")
                    nc.sync.dma_start(x_t[:, :], d_enc[128 * i : 128 * (i + 1), :])
                    stats = P_pw.tile([128, 6], F32, name="enc_st")
                    mv = P_pw.tile([128, 2], F32, name="enc_mv")
                    nc.vector.bn_stats(out=stats[:, :], in_=x_t[:, :])
                    nc.vector.bn_aggr(out=mv[:, :], in_=stats[:, :])
                    nc.scalar.activation(
                        out=mv[:, 1:2], in_=mv[:, 1:2], func=AF.Sqrt, bias=eps_t[:, :]
                    )
                    nc.vector.reciprocal(out=mv[:, 1:2], in_=mv[:, 1:2])
                    nc.vector.tensor_scalar(
                        out=enc_ln[i][:, :],
                        in0=x_t[:, :],
                        scalar1=mv[:, 0:1],
                        scalar2=mv[:, 1:2],
                        op0=ALU.subtract,
                        op1=ALU.mult,
                    )
                for i in range(4):
                    for j in range(4):
                        pt = PS_a.tile([128, 128], F32, name="tpose")
                        nc.tensor.transpose(
                            out=pt[:, :],
                            in_=enc_ln[i][:, 128 * j : 128 * (j + 1)],
                            identity=id128[:, :],
                        )
                        nc.vector.tensor_copy(
                            enc_Tb[j][:, 128 * i : 128 * (i + 1)], pt[:, :]
                        )

                # ----- projected keys pk_T (bf16) -----
                for g in range(KT):
                    pp = PS_g.tile([128, BS], F32, name="pk_ps", tag="mm_ps")
                    for k in range(KT):
                        nc.tensor.matmul(
                            pp[:, :],
                            lhsT=kwT[k][:, 128 * g : 128 * (g + 1)],
                            rhs=enc_Tb[k][:, :],
                            start=(k == 0),
                            stop=(k == KT - 1),
                        )
                    nc.vector.tensor_copy(pk_Tb[g][:, :], pp[:, :])

                # ----- embedding gather + LN + transpose -----
                xe_ln = [P_pre.tile([128, E], F32, name=f"xeln{i}") for i in range(2)]
                for i in range(2):
                    tgt_sb = P_pw.tile([128, 1], I32, name="tgt")
                    nc.sync.dma_start(tgt_sb[:, :], d_tgt[128 * i : 128 * (i + 1), :])
                    xg = P_pw.tile([128, E], F32, name="xemb")
                    nc.gpsimd.indirect_dma_start(
                        out=xg[:, :],
                        out_offset=None,
                        in_=d_emb[:, :],
                        in_offset=bass.IndirectOffsetOnAxis(ap=tgt_sb[:, 0:1], axis=0),
                    )
                    stats = P_pw.tile([128, 6], F32, name="xe_st")
                    mv = P_pw.tile([128, 2], F32, name="xe_mv")
                    nc.vector.bn_stats(out=stats[:, :], in_=xg[:, :])
                    nc.vector.bn_aggr(out=mv[:, :], in_=stats[:, :])
                    nc.scalar.activation(
                        out=mv[:, 1:2], in_=mv[:, 1:2], func=AF.Sqrt, bias=eps_t[:, :]
                    )
                    nc.vector.reciprocal(out=mv[:, 1:2], in_=mv[:, 1:2])
                    nc.vector.tensor_scalar(
                        out=xe_ln[i][:, :],
                        in0=xg[:, :],
                        scalar1=mv[:, 0:1],
                        scalar2=mv[:, 1:2],
                        op0=ALU.subtract,
                        op1=ALU.mult,
                    )
                xlnT = [P_pre.tile([128, TB], BF16, name=f"xlnT{k}") for k in range(2)]
                for i in range(2):
                    for j in range(2):
                        pt = PS_a.tile([128, 128], F32, name="tpose")
                        nc.tensor.transpose(
                            out=pt[:, :],
                            in_=xe_ln[i][:, 128 * j : 128 * (j + 1)],
                            identity=id128[:, :],
                        )
                        nc.vector.tensor_copy(
                            xlnT[j][:, 128 * i : 128 * (i + 1)], pt[:, :]
                        )
                ones_row = P_pre.tile([1, 128], BF16, name="ones")
                nc.vector.memset(ones_row[:, :], 1.0)
                if debug:
                    for k in range(2):
                        nc.sync.dma_start(dbg["dbg_xlnT"][128*k:128*(k+1), :], xlnT[k][:, :])

                # ----- x_gates = LN(emb[tgt]) @ w_x.T + b -----
                for m in range(2):
                    for b4 in range(4):
                        pp = PS_g.tile([128, 512], F32, name="xg_ps", tag="mm_ps")
                        for k in range(2):
                            nc.tensor.matmul(
                                pp[:, :],
                                lhsT=xlnT[k][:, 128 * m : 128 * (m + 1)],
                                rhs=xwT[k][:, 512 * b4 : 512 * (b4 + 1)],
                                start=(k == 0),
                                stop=False,
                            )
                        nc.tensor.matmul(
                            pp[:, :],
                            lhsT=ones_row[:, :],
                            rhs=xwTb[:, 512 * b4 : 512 * (b4 + 1)],
                            start=False,
                            stop=True,
                        )
                        nc.vector.tensor_copy(
                            x_gates[m][:, 512 * b4 : 512 * (b4 + 1)], pp[:, :]
                        )

            if debug:
                for k in range(KT):
                    nc.sync.dma_start(dbg["dbg_encT"][128*k:128*(k+1), :], enc_Tb[k][:, :])
                    nc.sync.dma_start(dbg["dbg_pkT"][128*k:128*(k+1), :], pk_Tb[k][:, :])
                    nc.sync.dma_start(dbg["dbg_h0T"][128*k:128*(k+1), :], h0_T[k][:, :].bitcast(F32))
                for m in range(2):
                    nc.sync.dma_start(dbg["dbg_xg"][128*m:128*(m+1), :], x_gates[m][:, :])
                nc.sync.dma_start(dbg["dbg_c0"][:, :], c_st[:, :])

            # ================= recurrence =================
            for t in range(T):
                if t == 0:
                    h_f32 = [h0_T[k][:, :] for k in range(KT)]
                    h_bf = [h0_Tb[k][:, :] for k in range(KT)]
                else:
                    h_f32 = [hs_T[k][:, BL * (t - 1) : BL * t] for k in range(KT)]
                    h_bf = [
                        hs_Tb[k][:, :].rearrange("p (b tt) -> p tt b", tt=T)[:, t - 1, :]
                        for k in range(KT)
                    ]

                # --- q_T = qw.T @ h (+qadd), bf16 ---
                q_Tb = [P_cell.tile([128, BL], BF16, name=f"qT{k}") for k in range(KT)]
                for g in range(KT):
                    pq = PS_a.tile([128, BL], F32, name="q_ps")
                    for k in range(KT):
                        nc.tensor.matmul(
                            pq[:, :],
                            lhsT=qwT[k][:, 128 * g : 128 * (g + 1)],
                            rhs=h_bf[k],
                            start=(k == 0),
                            stop=(k == KT - 1),
                        )
                    nc.vector.tensor_tensor(
                        out=q_Tb[g][:, :],
                        in0=pq[:, :],
                        in1=_bc_col(qadd[g][:, 0:1], BL),
                        op=ALU.add,
                    )

                # --- E = tanh(q + pk) (in-place tanh) ---
                esum = [P_cell.tile([128, BS], BF16, name=f"esum{g}") for g in range(KT)]
                for g in range(KT):
                    nc.vector.tensor_tensor(
                        out=esum[g][:, :].rearrange("p (b s) -> p b s", s=S),
                        in0=pk_Tb[g][:, :].rearrange("p (b s) -> p b s", s=S),
                        in1=_bc_free(q_Tb[g][:, :], S),
                        op=ALU.add,
                    )
                    nc.scalar.activation(
                        out=esum[g][:, :], in_=esum[g][:, :], func=AF.Tanh
                    )

                # --- energies = e_w . tanh -> [1, BS] psum ---
                pe = PS_a.tile([1, BS], F32, name="e_ps")
                for k in range(KT):
                    nc.tensor.matmul(
                        pe[:, :],
                        lhsT=ewT[k][:, :],
                        rhs=esum[k][:, :],
                        start=(k == 0),
                        stop=(k == KT - 1),
                    )

                # --- softmax over s (no max-subtraction; |e| bounded) ---
                expv = P_cell.tile([1, BS], F32, name="expv")
                nc.scalar.activation(out=expv[:, :], in_=pe[:, :], func=AF.Exp)
                ssum = P_cell.tile([1, BL], F32, name="ssum")
                nc.vector.tensor_reduce(
                    out=ssum[:, :],
                    in_=expv[:, :].rearrange("p (b s) -> p b s", s=S),
                    axis=mybir.AxisListType.X,
                    op=ALU.add,
                )
                nc.vector.reciprocal(out=ssum[:, :], in_=ssum[:, :])
                alpha = P_cell.tile([1, BS], BF16, name="alpha")
                nc.vector.tensor_tensor(
                    out=alpha[:, :].rearrange("p (b s) -> p b s", s=S),
                    in0=expv[:, :].rearrange("p (b s) -> p b s", s=S),
                    in1=_bc_free(ssum[:, :], S),
                    op=ALU.mult,
                )
                alpha_bc = P_cell.tile([128, BS], BF16, name="alpha_bc")
                nc.gpsimd.partition_broadcast(alpha_bc[:, :], alpha[:, :])
                if debug and t == 0:
                    for k in range(KT):
                        nc.sync.dma_start(dbg["dbg_qT"][128*k:128*(k+1), :], q_Tb[k][:, :])
                        nc.sync.dma_start(dbg["dbg_tanh"][128*k:128*(k+1), :], esum[k][:, :])
                    aex = P_cell.tile([1, BS], F32, name="aex")
                    nc.vector.tensor_copy(aex[:, :], alpha[:, :])
                    nc.sync.dma_start(dbg["dbg_alpha"][:, :], aex[:, :])

                # --- context_T[g, b] = sum_s alpha * enc_T ---
                ctx_T = [P_cell.tile([128, BL], F32R, name=f"ctxT{k}") for k in range(KT)]
                prod = P_cell.tile([128, BS], BF16, name="ctx_prod")
                for g in range(KT):
                    nc.vector.tensor_tensor(
                        out=prod[:, :],
                        in0=enc_Tb[g][:, :],
                        in1=alpha_bc[:, :],
                        op=ALU.mult,
                    )
                    with nc.allow_low_precision(reason="ctx f32r accum ok"):
                        nc.vector.tensor_reduce(
                            out=ctx_T[g][:, :],
                            in_=prod[:, :].rearrange("p (b s) -> p b s", s=S),
                            axis=mybir.AxisListType.X,
                            op=ALU.add,
                        )

                # --- gates = [ctx; h] @ w_z.T (f32r moving weights) + x_gates ---
                gates = P_cell.tile([BL, G4], F32, name="gates")
                xg_t = P_cell.tile([BL, G4], F32, name="xg_t")
                xrow = t % 16
                nc.sync.dma_start(
                    xg_t[:, :],
                    x_gates[t // 16][BL * xrow : BL * (xrow + 1), :],
                )
                for b4 in range(4):
                    pg = PS_g.tile([128, 512], F32, name="g_ps", tag="mm_ps")
                    for k in range(2 * KT):
                        lhs = ctx_T[k][:, :] if k < KT else h_f32[k - KT]
                        nc.tensor.matmul(
                            pg[0:BL, :],
                            lhsT=lhs,
                            rhs=wzT[k][:, 512 * b4 : 512 * (b4 + 1)],
                            start=(k == 0),
                            stop=(k == 2 * KT - 1),
                        )
                    nc.vector.tensor_tensor(
                        out=gates[:, 512 * b4 : 512 * (b4 + 1)],
                        in0=pg[0:BL, :],
                        in1=xg_t[:, 512 * b4 : 512 * (b4 + 1)],
                        op=ALU.add,
                    )

                if debug and t == 0:
                    for k in range(KT):
                        nc.sync.dma_start(dbg["dbg_ctxT"][128*k:128*(k+1), :], ctx_T[k][:, :].bitcast(F32))
                    nc.sync.dma_start(dbg["dbg_gates"][:, :], gates[:, :])
                # --- LSTM cell, in-place on gates slices ---
                g0, g1 = gates[:, 0:H], gates[:, H : 2 * H]
                g2, g3 = gates[:, 2 * H : 3 * H], gates[:, 3 * H : 4 * H]
                nc.scalar.activation(out=g0, in_=g0, func=AF.Sigmoid)
                nc.scalar.activation(out=g1, in_=g1, func=AF.Sigmoid)
                nc.scalar.activation(out=g2, in_=g2, func=AF.Tanh)
                nc.scalar.activation(out=g3, in_=g3, func=AF.Sigmoid)
                nc.vector.tensor_mul(g1, g1, c_st[:, :])  # sf*c
                nc.vector.tensor_mul(g0, g0, g2)  # si*tg
                nc.vector.tensor_add(c_st[:, :], g0, g1)  # c2
                nc.scalar.activation(out=g2, in_=c_st[:, :], func=AF.Tanh)
                h2 = P_cell.tile([BL, H], F32, name="h2")
                nc.vector.tensor_mul(h2[:, :], g3, g2)

                # --- transpose h2 -> hs_T / hs_Tb col t ---
                for k in range(KT):
                    pt = PS_a.tile([128, BL], F32, name="h_tpose")
                    nc.tensor.transpose(
                        out=pt[:, :],
                        in_=h2[:, 128 * k : 128 * (k + 1)],
                        identity=id8[:, :],
                    )
                    nc.vector.tensor_copy(hs_T[k][:, BL * t : BL * (t + 1)], pt[:, :])
                    hsb_v = hs_Tb[k][:, :].rearrange("p (b tt) -> p tt b", tt=T)
                    nc.vector.tensor_copy(hsb_v[:, t, :], pt[:, :])

            if debug:
                for k in range(KT):
                    nc.sync.dma_start(dbg["dbg_hsT"][128*k:128*(k+1), :], hs_T[k][:, :].bitcast(F32))
            # ================= AllGather hidden states =================
            for k in range(KT):
                nc.sync.dma_start(d_ccin[128 * k : 128 * (k + 1), :], hs_Tb[k][:, :])
            nc.gpsimd.collective_compute(
                "AllGather",
                ALU.bypass,
                replica_groups=[list(range(NCORES))],
                ins=[d_ccin[:, :]],
                outs=[d_ccout[:, :]],
            )

        # ================= projection phase =================
        with (
            tc.tile_pool(name="proj", bufs=1) as P_pj,
            tc.tile_pool(name="projw", bufs=3) as P_po,
            tc.tile_pool(name="psP", bufs=8, space="PSUM") as PS_p,
        ):
            hs_all = [P_pj.tile([128, BT], BF16, name=f"hsall{k}") for k in range(KT)]
            for k in range(KT):
                for r in range(NCORES):
                    nc.sync.dma_start(
                        hs_all[k][:, TB * r : TB * (r + 1)],
                        d_ccout[H * r + 128 * k : H * r + 128 * (k + 1), :],
                    )
            owT = [P_pj.tile([128, VL], BF16, name=f"owT{k}") for k in range(KT)]
            for k in range(KT):
                nc.sync.dma_start(owT[k][:, :], d_owT[128 * k : 128 * (k + 1), :])
            if debug:
                for k in range(KT):
                    nc.sync.dma_start(dbg["dbg_hsall"][128*k:128*(k+1), :], hs_all[k][:, :])
            ob_bc = P_pj.tile([128, VL], F32, name="ob_bc")
            ob_row = P_pj.tile([1, VL], F32, name="ob_row")
            nc.sync.dma_start(ob_row[:, :], d_ob[:, :])
            nc.gpsimd.partition_broadcast(ob_bc[:, :], ob_row[:, :])

            NV = 8  # vocab chunks
            VC = VL // NV  # 500
            for mt in range(BT // 128):
                for vc in range(NV):
                    pp = PS_p.tile([128, VC], F32, name="proj_ps")
                    for k in range(KT):
                        nc.tensor.matmul(
                            pp[:, :],
                            lhsT=hs_all[k][:, 128 * mt : 128 * (mt + 1)],
                            rhs=owT[k][:, VC * vc : VC * (vc + 1)],
                            start=(k == 0),
                            stop=(k == KT - 1),
                        )
                    ob_t = P_po.tile([128, VC], F32, name="proj_out")
                    nc.vector.tensor_tensor(
                        out=ob_t[:, :],
                        in0=pp[:, :],
                        in1=ob_bc[:, VC * vc : VC * (vc + 1)],
                        op=ALU.add,
                    )
                    if debug and mt == 0 and vc == 0:
                        nc.sync.dma_start(dbg["dbg_chunk"][:, :], ob_t[:, :])
                    # hs_all cols are (r, bl, t) => rows of out[(b t), v] are contiguous
                    dst = d_out[:, :, :].rearrange("b t v -> (b t) v")[
                        128 * mt : 128 * (mt + 1), VC * vc : VC * (vc + 1)
                    ]
                    nc.sync.dma_start(dst, ob_t[:, :])

    nc.compile()
    return nc


def _prep_inputs(inputs):
    """Host-side layout prep. Returns (in_maps, consts)."""
    f = lambda x: np.asarray(x, dtype=np.float32)
    targets = np.asarray(inputs["targets"])
    enc_hid = f(inputs["encoder_hidden"])
    enc_hn = f(inputs["enc_hn"])
    enc_cn = f(inputs["enc_cn"])
    emb = f(inputs["emb"])
    ln_enc_g = f(inputs["ln_enc_g"])
    ln_enc_b = f(inputs["ln_enc_b"])
    ln_emb_g = f(inputs["ln_emb_g"])
    ln_emb_b = f(inputs["ln_emb_b"])
    q_w = f(inputs["q_w"])
    q_b = f(inputs["q_b"])
    k_w = f(inputs["k_w"])
    e_w = f(inputs["e_w"])
    w_ih = f(inputs["w_ih"])
    w_hh = f(inputs["w_hh"])
    b_ih = f(inputs["b_ih"])
    b_hh = f(inputs["b_hh"])
    out_w = f(inputs["out_w"])
    out_b = f(inputs["out_b"])

    consts = dict(
        phw=[float(x) for x in f(inputs["proj_hn_w"])[0]],
        phb=float(f(inputs["proj_hn_b"])[0]),
        pcw=[float(x) for x in f(inputs["proj_cn_w"])[0]],
        pcb=float(f(inputs["proj_cn_b"])[0]),
    )

    # fold LN affines into adjacent matmuls
    kw_eff = k_w * ln_enc_g[None, :]
    qadd = q_b + k_w @ ln_enc_b
    w_ctx = w_ih[:, :H] * ln_enc_g[None, :]
    w_x = w_ih[:, H:] * ln_emb_g[None, :]
    b_gates = b_ih + b_hh + w_ih[:, :H] @ ln_enc_b + w_ih[:, H:] @ ln_emb_b

    w_zT = np.ascontiguousarray(
        np.concatenate([w_ctx.T, w_hh.T], axis=0), dtype=np.float32
    )
    x_wT = np.concatenate([w_x.T, b_gates[None, :]], axis=0)
    x_wT = np.ascontiguousarray(x_wT).astype(bf16)

    kwT_b = np.ascontiguousarray(kw_eff.T).astype(bf16)
    qwT_b = np.ascontiguousarray(q_w.T).astype(bf16)
    ewT_b = np.ascontiguousarray(e_w[0][:, None]).astype(bf16)
    qadd_c = np.ascontiguousarray(qadd[:, None], dtype=np.float32)

    in_maps = []
    for c in range(NCORES):
        bsl = slice(BL * c, BL * (c + 1))
        vs = slice(VL * c, VL * (c + 1))
        enc_c = np.ascontiguousarray(enc_hid[bsl].reshape(BS, H), dtype=np.float32)
        ehnT = np.ascontiguousarray(
            enc_hn[:, bsl].transpose(2, 0, 1).reshape(H, NL2 * BL), dtype=np.float32
        )
        ecn = np.ascontiguousarray(
            enc_cn[:, bsl].transpose(1, 0, 2).reshape(BL, NL2 * H), dtype=np.float32
        )
        tgt = np.ascontiguousarray(targets[bsl].T.reshape(TB, 1), dtype=np.int32)
        owT = np.ascontiguousarray(out_w[vs].T).astype(bf16)
        ob = np.ascontiguousarray(out_b[vs][None, :], dtype=np.float32)
        in_maps.append(
            {
                "enc": enc_c,
                "ehnT": ehnT,
                "ecn": ecn,
                "emb": emb,
                "tgt": tgt,
                "kwT": kwT_b,
                "qwT": qwT_b,
                "ewT": ewT_b,
                "qadd": qadd_c,
                "wzT": w_zT,
                "xwT": x_wT,
                "owT": owT,
                "ob": ob,
            }
        )
    return in_maps, consts


_CACHE = {}


def kernel(**inputs) -> np.ndarray:
    in_maps, consts = _prep_inputs(inputs)
    dbgf = bool(int(os.environ.get("KERNEL_DEBUG", "0")))
    key = (dbgf,) + tuple(consts["phw"] + consts["pcw"] + [consts["phb"], consts["pcb"]])
    if key not in _CACHE:
        _CACHE[key] = build_nc(consts, debug=dbgf)
    nc = _CACHE[key]
    res = run_bass_kernel_spmd(
        nc,
        in_maps,
        core_ids=list(range(NCORES)),
        trace=bool(int(os.environ.get("KERNEL_TRACE", "0"))),
    )
    kernel._last = res
    shards = [res.results[c]["out"] for c in range(NCORES)]
    return np.concatenate(shards, axis=2)


kernel._last = None


if __name__ == "__main__":
    shapes = {
        "targets": (B, T),
        "encoder_hidden": (B, S, H),
        "enc_hn": (NL2, B, H),
        "enc_cn": (NL2, B, H),
        "emb": (V, E),
        "ln_enc_g": (H,),
        "ln_enc_b": (H,),
        "ln_emb_g": (E,),
        "ln_emb_b": (E,),
        "proj_hn_w": (1, NL2),
        "proj_hn_b": (1,),
        "proj_cn_w": (1, NL2),
        "proj_cn_b": (1,),
        "q_w": (H, H),
        "q_b": (H,),
        "k_w": (H, H),
        "e_w": (1, H),
        "w_ih": (4 * H, H + E),
        "w_hh": (4 * H, H),
        "b_ih": (4 * H,),
        "b_hh": (4 * H,),
        "out_w": (V, H),
        "out_b": (V,),
    }
    dummy = {
        k: (
            np.zeros(s, np.int64)
            if k == "targets"
            else np.random.RandomState(0).randn(*s).astype(np.float32) * 0.1
        )
        for k, s in shapes.items()
    }
    _, consts = _prep_inputs(dummy)
    nc = build_nc(consts)
    print("build OK")



# revision 6
# speedup vs baseline: 1.0521x; 1.0521x over previous
"""AttentionDecoder Trainium2 kernel — 8-core SPMD, v2.

Strategy:
  - Data-parallel recurrence: core c owns batch slice [8c, 8c+8).
    LSTM+attention runs fully on-device in a single fused loop.
  - Hidden states kept in ONE bf16 transposed history buffer (t-major),
    with an extra t=-1 slot holding h0 (computed host-side along with c0
    from the tiny NL2-weight init projections).
  - Chunked AllGather: hs for t<16 is gathered at step 16 and the first
    half of the vocab projection is interleaved under steps 19..31; the
    second half runs as a short tail after step 31.
  - Vocab-parallel output projection: core c computes
    preds[:, :, 4000c:4000c+4000]; host concatenates the 8 vocab shards.
  - Engine balance: esum-add + alpha-broadcast + ctx-mult on GpSimd,
    reductions/copies on Vector, activations on Scalar, gates reordered
    to [i,f,o,g] so one sigmoid covers 3 gates.
"""

import os
import sys

sys.path.insert(0, "/opt/trn_rl_repo")

import ml_dtypes
import numpy as np

import concourse.bass as bass
from concourse import bacc
import concourse.mybir as mybir
import concourse.tile as tile
from concourse.bass_utils import run_bass_kernel_spmd
from concourse.masks import make_identity

# problem shapes (hardcoded per harness contract)
B, S, H, E, V, NL2, T = 64, 64, 512, 256, 32000, 4, 32
NCORES = 8
BL = B // NCORES  # 8 examples per core
VL = V // NCORES  # 4000 vocab rows per core
EPS = 1e-5
BS = BL * S  # 512 rows of encoder per core
TB = T * BL  # 256 (t, b) rows per core
G4 = 4 * H  # 2048 gate dim
BT = B * T  # 2048 gathered rows
KT = H // 128  # 4 partition tiles for the 512 hidden dim
KSTR = (T + 1) * BL  # 264: per-k stride in the hs history (slot 0 = h0)
TCH = T // 2  # 16 steps per projection chunk
CCH = TCH * BL  # 128 hs cols per chunk

F32 = mybir.dt.float32
F32R = mybir.dt.float32r
BF16 = mybir.dt.bfloat16
I32 = mybir.dt.int32
AF = mybir.ActivationFunctionType
ALU = mybir.AluOpType

bf16 = ml_dtypes.bfloat16


def _bc_free(ap, n):
    """Append a step-0 free dim of size n (broadcast along a new inner axis)."""
    return bass.AP(tensor=ap.tensor, offset=ap.offset, ap=[*ap.ap, [0, n]])


def _bc_col(ap, n):
    """[P, 1] column -> [P, n] broadcast (replace free dim with step-0)."""
    return bass.AP(tensor=ap.tensor, offset=ap.offset, ap=[ap.ap[0], [0, n]])


def build_nc():
    nc = bacc.Bacc()

    # ---------------- DRAM I/O ----------------
    d_enc = nc.dram_tensor("enc", [BS, H], F32, kind="ExternalInput")
    d_h0T = nc.dram_tensor("h0T", [H, BL], BF16, kind="ExternalInput")
    d_c0 = nc.dram_tensor("c0", [BL, H], F32, kind="ExternalInput")
    d_emb = nc.dram_tensor("emb", [V, E], F32, kind="ExternalInput")
    d_tgt = nc.dram_tensor("tgt", [TB, 1], I32, kind="ExternalInput")
    d_kwT = nc.dram_tensor("kwT", [H, H], BF16, kind="ExternalInput")
    d_qwT = nc.dram_tensor("qwT", [H, H], BF16, kind="ExternalInput")
    d_ewT = nc.dram_tensor("ewT", [H, 1], BF16, kind="ExternalInput")
    d_qadd = nc.dram_tensor("qadd", [128, KT], F32, kind="ExternalInput")
    d_wcT = nc.dram_tensor("wcT", [H, G4], F32R, kind="ExternalInput")
    d_whT = nc.dram_tensor("whT", [H, G4], BF16, kind="ExternalInput")
    d_xwT = nc.dram_tensor("xwT", [2 * 128 + 1, G4], BF16, kind="ExternalInput")
    d_owT = nc.dram_tensor("owT", [H, VL], BF16, kind="ExternalInput")
    d_ob = nc.dram_tensor("ob", [128, VL], BF16, kind="ExternalInput")
    d_out = nc.dram_tensor("out", [B, T, VL], F32, kind="ExternalOutput")

    # internal DRAM for the two chunked collectives
    d_ccin = [nc.dram_tensor(f"ccin{c}", [H, CCH], BF16) for c in range(2)]
    d_ccout = [
        nc.dram_tensor(f"ccout{c}", [NCORES * H, CCH], BF16, addr_space="Shared")
        for c in range(2)
    ]

    with tile.TileContext(nc) as tc:
        with (
            tc.tile_pool(name="persist", bufs=1) as P_per,
            tc.tile_pool(name="cell", bufs=2) as P_cell,
            tc.tile_pool(name="projout", bufs=4) as P_po,
        ):
            # ---------- persistent SBUF ----------
            id128 = P_per.tile([128, 128], F32, name="id128")
            make_identity(nc, id128[:, :])
            id8 = P_per.tile([8, 8], F32, name="id8")
            make_identity(nc, id8[:, :])
            eps_t = P_per.tile([128, 1], F32, name="eps")
            nc.vector.memset(eps_t[:, :], EPS)

            # bf16 transposed hidden history: col = k*KSTR + (t+1)*BL + b
            hs_Tb = P_per.tile([128, KT * KSTR], BF16, name="hsTb")
            c_st = P_per.tile([BL, H], F32, name="c_state")
            nc.sync.dma_start(c_st[:, :], d_c0[:, :])
            for k in range(KT):
                nc.sync.dma_start(
                    hs_Tb[:, k * KSTR : k * KSTR + BL],
                    d_h0T[128 * k : 128 * (k + 1), :],
                )

            # ---------- resident weights ----------
            kwT = [P_per.tile([128, H], BF16, name=f"kwT{k}") for k in range(KT)]
            qwT = [P_per.tile([128, H], BF16, name=f"qwT{k}") for k in range(KT)]
            ewT = [P_per.tile([128, 1], BF16, name=f"ewT{k}") for k in range(KT)]
            qadd4 = P_per.tile([128, KT], F32, name="qadd4")
            nc.sync.dma_start(qadd4[:, :], d_qadd[:, :])
            wcT = [P_per.tile([128, G4], F32R, name=f"wcT{k}") for k in range(KT)]
            whT = [P_per.tile([128, G4], BF16, name=f"whT{k}") for k in range(KT)]
            for k in range(KT):
                nc.sync.dma_start(kwT[k][:, :], d_kwT[128 * k : 128 * (k + 1), :])
                nc.sync.dma_start(qwT[k][:, :], d_qwT[128 * k : 128 * (k + 1), :])
                nc.sync.dma_start(ewT[k][:, :], d_ewT[128 * k : 128 * (k + 1), :])
                nc.sync.dma_start(wcT[k][:, :], d_wcT[128 * k : 128 * (k + 1), :])
                nc.sync.dma_start(whT[k][:, :], d_whT[128 * k : 128 * (k + 1), :])
            owT = [P_per.tile([128, VL], BF16, name=f"owT{k}") for k in range(KT)]
            for k in range(KT):
                nc.sync.dma_start(owT[k][:, :], d_owT[128 * k : 128 * (k + 1), :])
            ob_bc = P_per.tile([128, VL], BF16, name="ob_bc")
            nc.sync.dma_start(ob_bc[:, :], d_ob[:, :])

            ones_col = P_per.tile([1, 128], BF16, name="ones_col")
            nc.vector.memset(ones_col[:, :], 1.0)
            enc_Tb = [P_per.tile([128, BS], BF16, name=f"encT{k}") for k in range(KT)]
            pk_Tb = [P_per.tile([128, BS], BF16, name=f"pkT{g}") for g in range(KT)]
            x_gates = [P_per.tile([128, G4], BF16, name=f"xg{m}") for m in range(2)]
            hs_all = [P_per.tile([128, NCORES * CCH], BF16, name=f"hsall{k}")
                      for k in range(KT)]

            # ========== precompute (scoped pools; freed before recurrence) ====
            with (
                tc.tile_pool(name="pre", bufs=1) as P_pre,
                tc.tile_pool(name="prew", bufs=2) as P_pw,
                tc.tile_pool(name="psPre", bufs=3, space="PSUM") as PS_pre,
                tc.tile_pool(name="psPreMM", bufs=2, space="PSUM") as PS_pmm,
            ):
                xwT = [P_pre.tile([128, G4], BF16, name=f"xwT{k}") for k in range(2)]
                xwTb = P_pre.tile([1, G4], BF16, name="xwTbias")
                for k in range(2):
                    nc.sync.dma_start(xwT[k][:, :], d_xwT[128 * k : 128 * (k + 1), :])
                nc.sync.dma_start(xwTb[:, :], d_xwT[2 * 128 : 2 * 128 + 1, :])

                # ----- encoder LN (natural layout) + transpose -----
                enc_ln = [P_pre.tile([128, H], F32, name=f"encln{i}") for i in range(4)]
                for i in range(BS // 128):
                    x_t = P_pw.tile([128, H], F32, name="enc_in")
                    nc.sync.dma_start(x_t[:, :], d_enc[128 * i : 128 * (i + 1), :])
                    stats = P_pw.tile([128, 6], F32, name="enc_st")
                    mv = P_pw.tile([128, 2], F32, name="enc_mv")
                    nc.vector.bn_stats(out=stats[:, :], in_=x_t[:, :])
                    nc.vector.bn_aggr(out=mv[:, :], in_=stats[:, :])
                    nc.scalar.activation(
                        out=mv[:, 1:2], in_=mv[:, 1:2], func=AF.Sqrt, bias=eps_t[:, :]
                    )
                    nc.vector.reciprocal(out=mv[:, 1:2], in_=mv[:, 1:2])
                    nc.vector.tensor_scalar(
                        out=enc_ln[i][:, :],
                        in0=x_t[:, :],
                        scalar1=mv[:, 0:1],
                        scalar2=mv[:, 1:2],
                        op0=ALU.subtract,
                        op1=ALU.mult,
                    )
                for i in range(4):
                    for j in range(4):
                        pt = PS_pre.tile([128, 128], F32, name="tpose")
                        nc.tensor.transpose(
                            out=pt[:, :],
                            in_=enc_ln[i][:, 128 * j : 128 * (j + 1)],
                            identity=id128[:, :],
                        )
                        nc.vector.tensor_copy(
                            enc_Tb[j][:, 128 * i : 128 * (i + 1)], pt[:, :]
                        )

                # ----- projected keys pk_T (bf16) -----
                for g in range(KT):
                    pp = PS_pmm.tile([128, BS], F32, name="pk_ps")
                    for k in range(KT):
                        nc.tensor.matmul(
                            pp[:, :],
                            lhsT=kwT[k][:, 128 * g : 128 * (g + 1)],
                            rhs=enc_Tb[k][:, :],
                            start=(k == 0),
                            stop=(k == KT - 1),
                        )
                    nc.vector.tensor_copy(pk_Tb[g][:, :], pp[:, :])

                # ----- embedding gather + LN + transpose -----
                xe_ln = [P_pre.tile([128, E], F32, name=f"xeln{i}") for i in range(2)]
                for i in range(2):
                    tgt_sb = P_pw.tile([128, 1], I32, name="tgt")
                    nc.sync.dma_start(tgt_sb[:, :], d_tgt[128 * i : 128 * (i + 1), :])
                    xg = P_pw.tile([128, E], F32, name="xemb")
                    nc.gpsimd.indirect_dma_start(
                        out=xg[:, :],
                        out_offset=None,
                        in_=d_emb[:, :],
                        in_offset=bass.IndirectOffsetOnAxis(ap=tgt_sb[:, 0:1], axis=0),
                    )
                    stats = P_pw.tile([128, 6], F32, name="xe_st")
                    mv = P_pw.tile([128, 2], F32, name="xe_mv")
                    nc.vector.bn_stats(out=stats[:, :], in_=xg[:, :])
                    nc.vector.bn_aggr(out=mv[:, :], in_=stats[:, :])
                    nc.scalar.activation(
                        out=mv[:, 1:2], in_=mv[:, 1:2], func=AF.Sqrt, bias=eps_t[:, :]
                    )
                    nc.vector.reciprocal(out=mv[:, 1:2], in_=mv[:, 1:2])
                    nc.vector.tensor_scalar(
                        out=xe_ln[i][:, :],
                        in0=xg[:, :],
                        scalar1=mv[:, 0:1],
                        scalar2=mv[:, 1:2],
                        op0=ALU.subtract,
                        op1=ALU.mult,
                    )
                xlnT = [P_pre.tile([128, TB], BF16, name=f"xlnT{k}") for k in range(2)]
                for i in range(2):
                    for j in range(2):
                        pt = PS_pre.tile([128, 128], F32, name="tpose")
                        nc.tensor.transpose(
                            out=pt[:, :],
                            in_=xe_ln[i][:, 128 * j : 128 * (j + 1)],
                            identity=id128[:, :],
                        )
                        nc.vector.tensor_copy(
                            xlnT[j][:, 128 * i : 128 * (i + 1)], pt[:, :]
                        )
                ones_row = P_pre.tile([1, 128], BF16, name="ones")
                nc.vector.memset(ones_row[:, :], 1.0)

                # ----- x_gates = LN(emb[tgt]) @ w_x.T + b (iofg order) -----
                for m in range(2):
                    for b4 in range(4):
                        pp = PS_pmm.tile([128, 512], F32, name="xg_ps")
                        for k in range(2):
                            nc.tensor.matmul(
                                pp[:, :],
                                lhsT=xlnT[k][:, 128 * m : 128 * (m + 1)],
                                rhs=xwT[k][:, 512 * b4 : 512 * (b4 + 1)],
                                start=(k == 0),
                                stop=False,
                            )
                        nc.tensor.matmul(
                            pp[:, :],
                            lhsT=ones_row[:, :],
                            rhs=xwTb[:, 512 * b4 : 512 * (b4 + 1)],
                            start=False,
                            stop=True,
                        )
                        nc.vector.tensor_copy(
                            x_gates[m][:, 512 * b4 : 512 * (b4 + 1)], pp[:, :]
                        )

            # ============== recurrence + interleaved projection ==============
            with (
                tc.tile_pool(name="psSmall", bufs=3, space="PSUM") as PS_s,
                tc.tile_pool(name="psG", bufs=3, space="PSUM") as PS_g,
                tc.tile_pool(name="psProj", bufs=2, space="PSUM") as PS_p,
            ):
                NV = VL // 500  # 8 vocab chunks of 500

                def h_col(t):
                    # hs col base for h_{t} (slot t+1); k-tile k at + k*KSTR
                    return (t + 1) * BL

                def emit_proj_group(chunk, mt, vc):
                    pp = PS_p.tile([128, 500], F32, name="proj_ps")
                    for k in range(KT):
                        nc.tensor.matmul(
                            pp[:, :],
                            lhsT=hs_all[k][:, 128 * mt : 128 * (mt + 1)],
                            rhs=owT[k][:, 500 * vc : 500 * (vc + 1)],
                            start=(k == 0),
                            stop=(k == KT - 1),
                        )
                    ob_t = P_po.tile([128, 500], F32, name="proj_out")
                    nc.vector.tensor_tensor(
                        out=ob_t[:, :],
                        in0=pp[:, :],
                        in1=ob_bc[:, 500 * vc : 500 * (vc + 1)],
                        op=ALU.add,
                    )
                    # psum rows are (t, b) t-major for core mt's batch rows
                    dst = d_out[
                        BL * mt : BL * (mt + 1),
                        TCH * chunk : TCH * (chunk + 1),
                        500 * vc : 500 * (vc + 1),
                    ].rearrange("b t v -> t b v")
                    nc.sync.dma_start(dst, ob_t[:, :])

                def emit_gather(chunk):
                    base = CCH * chunk
                    for k in range(KT):
                        nc.sync.dma_start(
                            d_ccin[chunk][128 * k : 128 * (k + 1), :],
                            hs_Tb[:, k * KSTR + BL + base : k * KSTR + BL + base + CCH],
                        )
                    nc.gpsimd.collective_compute(
                        "AllGather",
                        ALU.bypass,
                        replica_groups=[list(range(NCORES))],
                        ins=[d_ccin[chunk][:, :]],
                        outs=[d_ccout[chunk][:, :]],
                    )
                    for r in range(NCORES):
                        for k in range(KT):
                            nc.sync.dma_start(
                                hs_all[k][:, CCH * r : CCH * (r + 1)],
                                d_ccout[chunk][
                                    H * r + 128 * k : H * r + 128 * (k + 1), :
                                ],
                            )

                # interleave schedule: (step -> list of (chunk, mt, vc))
                proj_sched = {}
                groups0 = [(0, mt, vc) for mt in range(NCORES) for vc in range(NV)]
                PROJ_START = 19
                per_step = -(-len(groups0) // (T - PROJ_START))  # ceil
                for i, grp in enumerate(groups0):
                    proj_sched.setdefault(PROJ_START + i // per_step, []).append(grp)

                for t in range(T):
                    hb = h_col(t - 1)

                    # --- q = qw.T @ h (+qadd): one PSUM bank, 4 copies ---
                    q_ps = PS_s.tile([128, KT * BL], F32, name="q_ps", tag="sm")
                    q_Tb = P_cell.tile([128, KT * BL], BF16, name="qT")
                    for g in range(KT):
                        for k in range(KT):
                            nc.tensor.matmul(
                                q_ps[:, BL * g : BL * (g + 1)],
                                lhsT=qwT[k][:, 128 * g : 128 * (g + 1)],
                                rhs=hs_Tb[:, k * KSTR + hb : k * KSTR + hb + BL],
                                start=(k == 0),
                                stop=(k == KT - 1),
                            )
                        nc.vector.tensor_tensor(
                            out=q_Tb[:, BL * g : BL * (g + 1)],
                            in0=q_ps[:, BL * g : BL * (g + 1)],
                            in1=_bc_col(qadd4[:, g : g + 1], BL),
                            op=ALU.add,
                        )

                    # --- gates h-part early (fills tensor while attention runs)
                    pg = []
                    for b4 in range(4):
                        pgb = PS_g.tile([128, 512], F32, name="g_ps", tag="gps")
                        pg.append(pgb)
                        for k in range(KT):
                            nc.tensor.matmul(
                                pgb[0:BL, :],
                                lhsT=hs_Tb[:, k * KSTR + hb : k * KSTR + hb + BL],
                                rhs=whT[k][:, 512 * b4 : 512 * (b4 + 1)],
                                start=(k == 0),
                                stop=False,
                            )

                    # --- E = tanh(q + pk); energies = e_w . E ---
                    e_ps = PS_s.tile([1, BS], F32, name="e_ps", tag="sm")
                    esum = [
                        P_cell.tile([128, BS], BF16, name=f"esum{g}", bufs=1) for g in range(KT)
                    ]
                    eeng = nc.vector if t in (TCH, TCH + 1, TCH + 2) else nc.gpsimd
                    for g in range(KT):
                        eeng.tensor_tensor(
                            out=esum[g][:, :].rearrange("p (b s) -> p b s", s=S),
                            in0=pk_Tb[g][:, :].rearrange("p (b s) -> p b s", s=S),
                            in1=_bc_free(q_Tb[:, BL * g : BL * (g + 1)], S),
                            op=ALU.add,
                        )
                        nc.scalar.activation(
                            out=esum[g][:, :], in_=esum[g][:, :], func=AF.Tanh
                        )
                        nc.tensor.matmul(
                            e_ps[:, :],
                            lhsT=ewT[g][:, :],
                            rhs=esum[g][:, :],
                            start=(g == 0),
                            stop=(g == KT - 1),
                        )

                    # --- softmax over s, normalization deferred to ctx ---
                    alpha = P_cell.tile([1, BS], BF16, name="alpha")
                    nc.scalar.activation(out=alpha[:, :], in_=e_ps[:, :], func=AF.Exp)
                    ssum = P_cell.tile([1, BL], F32, name="ssum")
                    nc.vector.tensor_reduce(
                        out=ssum[:, :],
                        in_=alpha[:, :].rearrange("p (b s) -> p b s", s=S),
                        axis=mybir.AxisListType.X,
                        op=ALU.add,
                    )
                    nc.vector.reciprocal(out=ssum[:, :], in_=ssum[:, :])
                    nc.vector.tensor_tensor(
                        out=alpha[:, :].rearrange("p (b s) -> p b s", s=S),
                        in0=alpha[:, :].rearrange("p (b s) -> p b s", s=S),
                        in1=_bc_free(ssum[:, :], S),
                        op=ALU.mult,
                    )
                    # broadcast alpha to 128 partitions via rank-1 matmul
                    abc_ps = PS_s.tile([128, BS], F32, name="abc_ps", tag="sm")
                    nc.tensor.matmul(
                        abc_ps[:, :],
                        lhsT=ones_col[:, :],
                        rhs=alpha[:, :],
                        start=True,
                        stop=True,
                    )
                    alpha_bc = P_cell.tile([128, BS], BF16, name="alpha_bc", bufs=1)
                    nc.vector.tensor_copy(alpha_bc[:, :], abc_ps[:, :])

                    # --- context_T[g, b] = (sum_s exp * enc_T) / ssum ---
                    ctx32 = P_cell.tile([128, KT * BL], F32R, name="ctx32")
                    veng = nc.vector if t in (TCH, TCH + 1, TCH + 2) else nc.gpsimd
                    for g in range(KT):
                        prod = P_cell.tile(
                            [128, BS], BF16, name="ctx_prod", tag="prod", bufs=2
                        )
                        veng.tensor_tensor(
                            out=prod[:, :],
                            in0=enc_Tb[g][:, :],
                            in1=alpha_bc[:, :],
                            op=ALU.mult,
                        )
                        with nc.allow_low_precision(reason="ctx f32r accum ok"):
                            nc.vector.tensor_reduce(
                                out=ctx32[:, BL * g : BL * (g + 1)],
                                in_=prod[:, :].rearrange("p (b s) -> p b s", s=S),
                                axis=mybir.AxisListType.X,
                                op=ALU.add,
                            )

                    # --- gates ctx-part + x_gates add ---
                    gates = P_cell.tile([BL, G4], F32, name="gates", bufs=1)
                    xg_t = P_cell.tile([BL, G4], BF16, name="xg_t", bufs=3)
                    xrow = BL * (t % TCH)
                    nc.sync.dma_start(
                        xg_t[:, :], x_gates[t // TCH][xrow : xrow + BL, :]
                    )
                    for b4 in range(4):
                        for k in range(KT):
                            nc.tensor.matmul(
                                pg[b4][0:BL, :],
                                lhsT=ctx32[:, BL * k : BL * (k + 1)],
                                rhs=wcT[k][:, 512 * b4 : 512 * (b4 + 1)],
                                start=False,
                                stop=(k == KT - 1),
                            )
                        nc.vector.tensor_tensor(
                            out=gates[:, 512 * b4 : 512 * (b4 + 1)],
                            in0=pg[b4][0:BL, :],
                            in1=xg_t[:, 512 * b4 : 512 * (b4 + 1)],
                            op=ALU.add,
                        )

                    # --- LSTM cell (iofg order: g0=i, g1=f, g2=o, g3=g~) ---
                    g_i = gates[:, 0:H]
                    g_f = gates[:, H : 2 * H]
                    g_o = gates[:, 2 * H : 3 * H]
                    g_g = gates[:, 3 * H : 4 * H]
                    nc.scalar.activation(
                        out=gates[:, 0 : 3 * H], in_=gates[:, 0 : 3 * H],
                        func=AF.Sigmoid,
                    )
                    nc.scalar.activation(out=g_g, in_=g_g, func=AF.Tanh)
                    nc.vector.tensor_mul(g_f, g_f, c_st[:, :])  # f*c
                    nc.vector.tensor_mul(g_i, g_i, g_g)  # i*g~
                    nc.vector.tensor_add(c_st[:, :], g_i, g_f)  # c2
                    nc.scalar.activation(out=g_g, in_=c_st[:, :], func=AF.Tanh)
                    h2 = P_cell.tile([BL, H], F32, name="h2", bufs=1)
                    nc.vector.tensor_mul(h2[:, :], g_o, g_g)

                    # --- transpose h2 into history (one bank, one copy) ---
                    htp = PS_s.tile([128, KT * BL], F32, name="htp", tag="sm")
                    for k in range(KT):
                        nc.tensor.transpose(
                            out=htp[:, BL * k : BL * (k + 1)],
                            in_=h2[:, 128 * k : 128 * (k + 1)],
                            identity=id8[:, :],
                        )
                    dst = hs_Tb[:, :].rearrange(
                        "p (k t b) -> p k t b", k=KT, b=BL
                    )[:, :, t + 1, :]
                    nc.vector.tensor_copy(
                        dst, htp[:, :].rearrange("p (k b) -> p k b", b=BL)
                    )

                    # --- interleaved projection work ---
                    if t == TCH - 1:
                        emit_gather(0)
                    for grp in proj_sched.get(t, []):
                        emit_proj_group(*grp)

                # ---- tail: second chunk ----
                emit_gather(1)
                for mt in range(NCORES):
                    for vc in range(NV):
                        emit_proj_group(1, mt, vc)

    nc.compile()
    return nc


def _prep_inputs(inputs):
    """Host-side layout prep. Returns per-core input maps."""
    f = lambda x: np.asarray(x, dtype=np.float32)
    targets = np.asarray(inputs["targets"])
    enc_hid = f(inputs["encoder_hidden"])
    enc_hn = f(inputs["enc_hn"])
    enc_cn = f(inputs["enc_cn"])
    emb = f(inputs["emb"])
    ln_enc_g = f(inputs["ln_enc_g"])
    ln_enc_b = f(inputs["ln_enc_b"])
    ln_emb_g = f(inputs["ln_emb_g"])
    ln_emb_b = f(inputs["ln_emb_b"])
    q_w = f(inputs["q_w"])
    q_b = f(inputs["q_b"])
    k_w = f(inputs["k_w"])
    e_w = f(inputs["e_w"])
    w_ih = f(inputs["w_ih"])
    w_hh = f(inputs["w_hh"])
    b_ih = f(inputs["b_ih"])
    b_hh = f(inputs["b_hh"])
    out_w = f(inputs["out_w"])
    out_b = f(inputs["out_b"])

    # h0/c0: tiny NL2-weight linear combos, done on host
    phw = f(inputs["proj_hn_w"])[0]
    phb = float(f(inputs["proj_hn_b"])[0])
    pcw = f(inputs["proj_cn_w"])[0]
    pcb = float(f(inputs["proj_cn_b"])[0])
    h0 = np.einsum("lbh,l->bh", enc_hn, phw) + phb  # [B, H]
    c0 = np.einsum("lbh,l->bh", enc_cn, pcw) + pcb  # [B, H]

    # fold LN affines into adjacent matmuls
    kw_eff = k_w * ln_enc_g[None, :]
    qadd = q_b + k_w @ ln_enc_b
    w_ctx = w_ih[:, :H] * ln_enc_g[None, :]
    w_x = w_ih[:, H:] * ln_emb_g[None, :]
    b_gates = b_ih + b_hh + w_ih[:, :H] @ ln_enc_b + w_ih[:, H:] @ ln_emb_b

    # reorder gate blocks [i, f, g, o] -> [i, f, o, g]
    perm = np.r_[0:H, H : 2 * H, 3 * H : 4 * H, 2 * H : 3 * H]
    w_ctx, w_x, w_hh_p = w_ctx[perm], w_x[perm], w_hh[perm]
    b_gates = b_gates[perm]

    wcT = np.ascontiguousarray(w_ctx.T, dtype=np.float32)
    whT = np.ascontiguousarray(w_hh_p.T).astype(bf16)
    x_wT = np.concatenate([w_x.T, b_gates[None, :]], axis=0)
    x_wT = np.ascontiguousarray(x_wT).astype(bf16)

    kwT_b = np.ascontiguousarray(kw_eff.T).astype(bf16)
    qwT_b = np.ascontiguousarray(q_w.T).astype(bf16)
    ewT_b = np.ascontiguousarray(e_w[0][:, None]).astype(bf16)
    qadd4 = np.ascontiguousarray(qadd.reshape(KT, 128).T, dtype=np.float32)

    in_maps = []
    for c in range(NCORES):
        bsl = slice(BL * c, BL * (c + 1))
        vs = slice(VL * c, VL * (c + 1))
        enc_c = np.ascontiguousarray(enc_hid[bsl].reshape(BS, H), dtype=np.float32)
        tgt = np.ascontiguousarray(targets[bsl].T.reshape(TB, 1), dtype=np.int32)
        h0T = np.ascontiguousarray(h0[bsl].T).astype(bf16)
        c0_c = np.ascontiguousarray(c0[bsl], dtype=np.float32)
        owT = np.ascontiguousarray(out_w[vs].T).astype(bf16)
        ob = np.ascontiguousarray(np.broadcast_to(out_b[vs].astype(bf16), (128, VL)))
        in_maps.append(
            {
                "enc": enc_c,
                "h0T": h0T,
                "c0": c0_c,
                "emb": emb,
                "tgt": tgt,
                "kwT": kwT_b,
                "qwT": qwT_b,
                "ewT": ewT_b,
                "qadd": qadd4,
                "wcT": wcT,
                "whT": whT,
                "xwT": x_wT,
                "owT": owT,
                "ob": ob,
            }
        )
    return in_maps


_CACHE = {}


def kernel(**inputs) -> np.ndarray:
    in_maps = _prep_inputs(inputs)
    if "nc" not in _CACHE:
        _CACHE["nc"] = build_nc()
    nc = _CACHE["nc"]
    res = run_bass_kernel_spmd(
        nc,
        in_maps,
        core_ids=list(range(NCORES)),
        trace=bool(int(os.environ.get("KERNEL_TRACE", "0"))),
    )
    kernel._last = res
    shards = [res.results[c]["out"] for c in range(NCORES)]
    return np.concatenate(shards, axis=2)


kernel._last = None


if __name__ == "__main__":
    nc = build_nc()
    print("build OK")
